# revision 5
# baseline (speedup 1.0000x reference)
"""Single-head attention (B=8, S=2048, E=1024, H=128) with softmax + deterministic
dropout, data-parallel over batch across 8 NeuronCores (one batch element per core).

Per-core layout strategy ("transposed attention"):
  - host ships xT = x[b].T           [E, S]  (so contraction dim E is on partitions)
  - host ships keepT = keep[b].T     [S, S]  (dropout mask, t-major)
  - qT/kT/vT[h, s] = w.T @ xT        (PE, N=512 tiles, accumulate over 8 E-chunks)
  - v natural [t, h] via 16 PE transposes of vT
  - attT[t, s] = k[t-chunk] @ qT     (PE; lhsT = kT chunk, rhs = qT slice)
  - expT = exp(attT * E^-0.5)        (ACT, reads PSUM directly)
  - denomT[1, s] += ones.T @ expT    (PE, M=1 matmul, accumulated over t-chunks)
  - attd = expT * keepT              (DVE)
  - outT[h, s] += v[t-chunk].T @ attd  (PE, accumulated over t-chunks)
  - normalize by 1/denom fused into the final PSUM->SBUF copy (ACT per-partition
    scale) after PE-transposing outT back to natural [s, h] layout.
"""

import sys

for _p in ("/opt/trn_rl_repo",):
    if _p not in sys.path:
        sys.path.append(_p)

import numpy as np

B, S, E, H = 8, 2048, 1024, 128
DROP_P = 0.1
P = 128
NT = S // P  # 16 t-chunks
NE = E // P  # 8 e-chunks
SG = 512     # s-group width (one fp32 PSUM bank)
NSG = S // SG  # 4
NC4 = SG // P  # 4 128-chunks per s-group

_program_cache = {}


def _build_program():
    if "nc" in _program_cache:
        return _program_cache["nc"]

    import concourse.bass as bass  # noqa: F401
    import concourse.mybir as mybir
    import concourse.tile as tile
    from concourse import bacc
    from concourse.masks import make_identity

    f32 = mybir.dt.float32
    Exp = mybir.ActivationFunctionType.Exp
    Copy = mybir.ActivationFunctionType.Copy
    SCALE = float(E) ** -0.5

    nc = bacc.Bacc("TRN2", target_bir_lowering=False, debug=False)
    xT_d = nc.dram_tensor("xT", [E, S], f32, kind="ExternalInput").ap()
    keepT_d = nc.dram_tensor("keepT", [S, S], f32, kind="ExternalInput").ap()
    wq_d = nc.dram_tensor("wq", [E, H], f32, kind="ExternalInput").ap()
    wk_d = nc.dram_tensor("wk", [E, H], f32, kind="ExternalInput").ap()
    wv_d = nc.dram_tensor("wv", [E, H], f32, kind="ExternalInput").ap()
    out_d = nc.dram_tensor("out", [S, H], f32, kind="ExternalOutput").ap()

    xT_r = xT_d.rearrange("(eo p) s -> p eo s", p=P)
    w_rs = [w.rearrange("(eo p) h -> p eo h", p=P) for w in (wq_d, wk_d, wv_d)]

    with tile.TileContext(nc) as tc:
        with (
            tc.tile_pool(name="consts", bufs=1) as consts,
            tc.tile_pool(name="xw", bufs=1) as xw_pool,
            tc.tile_pool(name="qkv", bufs=1) as qkv_pool,
        ):
            identity = consts.tile([P, P], f32)
            make_identity(nc, identity)
            ones_t = consts.tile([P, 1], f32)
            nc.vector.memset(ones_t, 1.0)

            # -------- load x^T and weights --------
            xT_sb = xw_pool.tile([P, NE, S], f32)
            for e in range(NE):
                nc.sync.dma_start(xT_sb[:, e, :], xT_r[:, e, :])
            w_sb = xw_pool.tile([P, 3, NE, H], f32)
            for j in range(3):
                for e in range(NE):
                    nc.sync.dma_start(w_sb[:, j, e, :], w_rs[j][:, e, :])

            # -------- projections: qT/kT/vT [H, S] --------
            qkvT_sb = qkv_pool.tile([P, 3, S], f32)  # [h, (q|k|v), s]
            v_sb = qkv_pool.tile([P, NT, H], f32)    # v natural: [t_in, t_chunk, h]
            with (
                tc.tile_pool(name="proj_ps", bufs=3, space="PSUM") as proj_ps,
                tc.tile_pool(name="tr_ps", bufs=2, space="PSUM") as tr_ps,
            ):
                for j in range(3):
                    for c in range(S // SG):
                        ps = proj_ps.tile([P, SG], f32, tag="proj")
                        for e in range(NE):
                            nc.tensor.matmul(
                                ps,
                                w_sb[:, j, e, :],
                                xT_sb[:, e, c * SG:(c + 1) * SG],
                                start=(e == 0),
                                stop=(e == NE - 1),
                            )
                        nc.any.tensor_copy(qkvT_sb[:, j, c * SG:(c + 1) * SG], ps)
                # v natural via PE transpose of vT
                for t in range(NT):
                    ps = tr_ps.tile([P, P], f32, tag="vtr")
                    nc.tensor.transpose(
                        ps, qkvT_sb[:, 2, t * P:(t + 1) * P], identity
                    )
                    nc.any.tensor_copy(v_sb[:, t, :], ps)

            # -------- main attention loop over s-groups --------
            with (
                tc.tile_pool(name="att_ps", bufs=2, space="PSUM") as att_ps,
                tc.tile_pool(name="out_ps", bufs=2, space="PSUM") as out_ps,
                tc.tile_pool(name="den_ps", bufs=2, space="PSUM") as den_ps,
                tc.tile_pool(name="tr2_ps", bufs=2, space="PSUM") as tr2_ps,
                tc.tile_pool(name="sb", bufs=3) as sb_pool,
                tc.tile_pool(name="sb2", bufs=2) as sb2_pool,
            ):
                for sg in range(NSG):
                    s_lo = sg * SG
                    s_sl = slice(s_lo, s_lo + SG)
                    psum_out = out_ps.tile([P, SG], f32, tag="out")
                    psum_den = den_ps.tile([1, SG], f32, tag="den")
                    for t in range(NT):
                        psum_att = att_ps.tile([P, SG], f32, tag="att")
                        nc.tensor.matmul(
                            psum_att,
                            qkvT_sb[:, 1, t * P:(t + 1) * P],  # kT chunk [H, 128]
                            qkvT_sb[:, 0, s_sl],               # qT slice [H, 512]
                            start=True,
                            stop=True,
                        )
                        expT = sb_pool.tile([P, SG], f32, tag="exp")
                        nc.scalar.activation(expT, psum_att, Exp, scale=SCALE)
                        nc.tensor.matmul(
                            psum_den,
                            ones_t,
                            expT,
                            start=(t == 0),
                            stop=(t == NT - 1),
                        )
                        keep_sb = sb_pool.tile([P, SG], f32, tag="keep")
                        nc.sync.dma_start(
                            keep_sb, keepT_d[t * P:(t + 1) * P, s_sl]
                        )
                        attd = sb_pool.tile([P, SG], f32, tag="attd")
                        nc.vector.tensor_mul(out=attd, in0=expT, in1=keep_sb)
                        nc.tensor.matmul(
                            psum_out,
                            v_sb[:, t, :],
                            attd,
                            start=(t == 0),
                            stop=(t == NT - 1),
                        )

                    # denominator -> natural-layout reciprocal [s_in, 1] chunks
                    den_sb = sb2_pool.tile([1, SG], f32, tag="den_sb")
                    nc.any.tensor_copy(den_sb, psum_den)
                    outT_sb = sb2_pool.tile([P, SG], f32, tag="outT")
                    nc.any.tensor_copy(outT_sb, psum_out)
                    recip_nat = sb2_pool.tile([P, NC4], f32, tag="recip")
                    for c in range(NC4):
                        ps_rt = tr2_ps.tile([P, P], f32, tag="tr", name="ps_rt")
                        ps_r = ps_rt[:, 0:1]
                        nc.tensor.transpose(
                            ps_r, den_sb[:, c * P:(c + 1) * P], identity[0:1, 0:1]
                        )
                        nc.vector.reciprocal(recip_nat[:, c:c + 1], ps_r)
                    # transpose outT back to natural [s, h] and scale by 1/denom
                    for c in range(NC4):
                        ps_o = tr2_ps.tile([P, P], f32, tag="tr")
                        nc.tensor.transpose(
                            ps_o, outT_sb[:, c * P:(c + 1) * P], identity
                        )
                        out_nat = sb2_pool.tile([P, H], f32, tag="out_nat")
                        nc.scalar.activation(
                            out_nat, ps_o, Copy, scale=recip_nat[:, c:c + 1]
                        )
                        row = s_lo + c * P
                        nc.sync.dma_start(out_d[row:row + P, :], out_nat)

    nc.compile()
    _program_cache["nc"] = nc
    return nc


def kernel(x, wq, wk, wv, drop_u):
    from concourse import bass_utils

    x = np.asarray(x)
    wq = np.asarray(wq)
    wk = np.asarray(wk)
    wv = np.asarray(wv)
    drop_u = np.asarray(drop_u)

    nc = _build_program()
    inv_keep = np.float32(1.0) / np.float32(1.0 - DROP_P)
    in_maps = []
    for b in range(B):
        xT = np.ascontiguousarray(x[b].T)
        keepT = (drop_u[b].T >= np.float32(DROP_P)).astype(np.float32) * inv_keep
        keepT = np.ascontiguousarray(keepT)
        in_maps.append(
            {"xT": xT, "keepT": keepT, "wq": np.asarray(wq), "wk": np.asarray(wk),
             "wv": np.asarray(wv)}
        )
    res = bass_utils.run_bass_kernel_spmd(
        nc, in_maps, core_ids=list(range(B)), trace=False
    )
    return np.stack([res.results[b]["out"] for b in range(B)], axis=0)


# revision 9
# speedup vs baseline: 2.2746x; 2.2746x over previous
"""Single-head attention (B=8, S=2048, E=1024, H=128) with softmax + deterministic
dropout, data-parallel over batch across 8 NeuronCores (one batch element per core).

Per-core layout strategy ("transposed attention"):
  - host ships xT = x[b].T           [E, S]  (contraction dim E on partitions)
  - host ships keepT = keep[b].T     [S, S]  fp16 {0,1} (dropout mask, t-major)
  - qT/kT/vT[h, s] = w.T @ xT        (PE, f32r operands, fp32 PSUM)
  - v natural [t, h] via 16 PE transposes of vT (fp16)
  - attT[t, s] = k[t-chunk] @ qT     (PE, f32r; lhsT = kT chunk, rhs = qT slice)
  - expT = exp(attT * E^-0.5)        (ACT, PSUM -> fp16 SBUF)
  - denomT[1, s] += ones.T @ expT    (PE fp16, M=1, accumulated over t-chunks)
  - attd = expT * keepT              (DVE fp16, 2x mode)
  - outT[h, s] += v[t-chunk].T @ attd  (PE fp16, fp32 PSUM accumulation)
  - normalize by 1/(0.9*denom) fused into the final PSUM->SBUF copy (ACT
    per-partition scale) after PE-transposing outT back to natural [s, h].

Matmul dtypes: float32r (TF32-like, ~1e-4) feeds the softmax logits, whose
absolute scale is ~0.1, so the logit error is ~1e-5 in exp units; fp16 in the
value path contributes ~1.5e-4. End-to-end output error ~2.5e-4 L2 vs fp32.
"""

import sys

for _p in ("/opt/trn_rl_repo",):
    if _p not in sys.path:
        sys.path.append(_p)

import numpy as np

B, S, E, H = 8, 2048, 1024, 128
DROP_P = 0.1
P = 128

_program_cache = {}


def _build_program(S=S, E=E):
    key = (S, E)
    if key in _program_cache:
        return _program_cache[key]
    NT = S // P  # t-chunks
    NE = E // P  # e-chunks
    SG = 512     # s-group width (one fp32 PSUM bank)
    NSG = S // SG
    NC4 = SG // P

    import concourse.bass as bass  # noqa: F401
    import concourse.mybir as mybir
    import concourse.tile as tile
    from concourse import bacc
    from concourse.masks import make_identity

    f32 = mybir.dt.float32
    f32r = mybir.dt.float32r
    f16 = mybir.dt.float16
    Exp = mybir.ActivationFunctionType.Exp
    Copy = mybir.ActivationFunctionType.Copy
    SCALE = float(E) ** -0.5

    nc = bacc.Bacc("TRN2", target_bir_lowering=False, debug=False)
    xT_d = nc.dram_tensor("xT", [E, S], f32r, kind="ExternalInput").ap()
    keepT_d = nc.dram_tensor("keepT", [S, S], f16, kind="ExternalInput").ap()
    wq_d = nc.dram_tensor("wq", [E, H], f32r, kind="ExternalInput").ap()
    wk_d = nc.dram_tensor("wk", [E, H], f32r, kind="ExternalInput").ap()
    wv_d = nc.dram_tensor("wv", [E, H], f32r, kind="ExternalInput").ap()
    out_d = nc.dram_tensor("out", [S, H], f32, kind="ExternalOutput").ap()

    xT_r = xT_d.rearrange("(eo p) s -> p eo s", p=P)
    w_rs = [w.rearrange("(eo p) h -> p eo h", p=P) for w in (wq_d, wk_d, wv_d)]

    with tile.TileContext(nc) as tc:
        with (
            tc.tile_pool(name="consts", bufs=1) as consts,
            tc.tile_pool(name="xw", bufs=1) as xw_pool,
            tc.tile_pool(name="qkv", bufs=1) as qkv_pool,
        ):
            identity = consts.tile([P, P], f32)
            make_identity(nc, identity)
            identity16 = consts.tile([P, P], f16)
            nc.any.tensor_copy(identity16, identity)
            ones_t = consts.tile([P, 1], f16)
            nc.vector.memset(ones_t, 1.0)

            # -------- load x^T and weights --------
            xT_sb = xw_pool.tile([P, NE, S], f32r)
            for e in range(NE):
                nc.sync.dma_start(xT_sb[:, e, :], xT_r[:, e, :])
            w_sb = xw_pool.tile([P, 3, NE, H], f32r)
            for j in range(3):
                for e in range(NE):
                    nc.sync.dma_start(w_sb[:, j, e, :], w_rs[j][:, e, :])

            # -------- projections: qT/kT [H, S] f32r; vT -> v natural f16 --------
            qkT_sb = qkv_pool.tile([P, 2, S], f32r)  # [h, (q|k), s]
            vT_sb = qkv_pool.tile([P, S], f16)
            v_sb = qkv_pool.tile([P, NT, H], f16)    # v natural: [t_in, t_chunk, h]
            with (
                tc.tile_pool(name="proj_ps", bufs=3, space="PSUM") as proj_ps,
                tc.tile_pool(name="tr_ps", bufs=2, space="PSUM") as tr_ps,
            ):
                for j in range(3):
                    for c in range(S // SG):
                        ps = proj_ps.tile([P, SG], f32, tag="proj")
                        for e in range(NE):
                            nc.tensor.matmul(
                                ps,
                                w_sb[:, j, e, :],
                                xT_sb[:, e, c * SG:(c + 1) * SG],
                                start=(e == 0),
                                stop=(e == NE - 1),
                            )
                        if j < 2:
                            nc.any.tensor_copy(qkT_sb[:, j, c * SG:(c + 1) * SG], ps)
                        else:
                            nc.any.tensor_copy(vT_sb[:, c * SG:(c + 1) * SG], ps)
                # v natural via PE transpose of vT (fp16, exact for fp16 data)
                for t in range(NT):
                    ps_v = tr_ps.tile([P, P], f16, tag="vtr")
                    nc.tensor.transpose(
                        ps_v, vT_sb[:, t * P:(t + 1) * P], identity16
                    )
                    nc.any.tensor_copy(v_sb[:, t, :], ps_v)

            # -------- main attention loop over s-groups --------
            with (
                tc.tile_pool(name="att_ps", bufs=3, space="PSUM") as att_ps,
                tc.tile_pool(name="out_ps", bufs=2, space="PSUM") as out_ps,
                tc.tile_pool(name="den_ps", bufs=1, space="PSUM") as den_ps,
                tc.tile_pool(name="tr2_ps", bufs=2, space="PSUM") as tr2_ps,
                tc.tile_pool(name="sb", bufs=3) as sb_pool,
                tc.tile_pool(name="sb2", bufs=2) as sb2_pool,
            ):
                for sg in range(NSG):
                    s_lo = sg * SG
                    s_sl = slice(s_lo, s_lo + SG)
                    psum_out = out_ps.tile([P, SG], f32, tag="out")
                    psum_den = den_ps.tile([1, SG], f32, tag="den")
                    expTs = {}
                    attds = {}

                    def emit_front(t, s_sl=s_sl, expTs=expTs, attds=attds):
                        psum_att = att_ps.tile([P, SG], f32, tag="att", name=f"att{t}")
                        nc.tensor.matmul(
                            psum_att,
                            qkT_sb[:, 1, t * P:(t + 1) * P],  # kT chunk [H, 128]
                            qkT_sb[:, 0, s_sl],               # qT slice [H, 512]
                            start=True,
                            stop=True,
                        )
                        expT = sb_pool.tile([P, SG], f16, tag="exp", name=f"exp{t}")
                        nc.scalar.activation(expT, psum_att, Exp, scale=SCALE)
                        keep_sb = sb_pool.tile([P, SG], f16, tag="keep", name=f"keep{t}")
                        nc.sync.dma_start(keep_sb, keepT_d[t * P:(t + 1) * P, s_sl])
                        attd = sb_pool.tile([P, SG], f16, tag="attd", name=f"attd{t}")
                        nc.vector.tensor_mul(out=attd, in0=expT, in1=keep_sb)
                        expTs[t] = expT
                        attds[t] = attd

                    def emit_back(t, psum_den=psum_den, psum_out=psum_out,
                                  expTs=expTs, attds=attds):
                        nc.tensor.matmul(
                            psum_den,
                            ones_t,
                            expTs.pop(t),
                            start=(t == 0),
                            stop=(t == NT - 1),
                        )
                        nc.tensor.matmul(
                            psum_out,
                            v_sb[:, t, :],
                            attds.pop(t),
                            start=(t == 0),
                            stop=(t == NT - 1),
                        )

                    # software pipeline: back-stage ops run one iteration behind
                    # the att matmul so PE never waits on ACT/DVE results.
                    for t in range(NT):
                        emit_front(t)
                        if t >= 1:
                            emit_back(t - 1)
                    emit_back(NT - 1)

                    # denominator -> natural-layout 1/(0.9*den) chunks [s_in, 1]
                    den_sb = sb2_pool.tile([1, SG], f32, tag="den_sb")
                    nc.scalar.mul(den_sb, psum_den, 1.0 - DROP_P)
                    outT_sb = sb2_pool.tile([P, SG], f32, tag="outT")
                    nc.any.tensor_copy(outT_sb, psum_out)
                    recip_nat = sb2_pool.tile([P, NC4], f32, tag="recip")
                    for c in range(NC4):
                        ps_rt = tr2_ps.tile([P, P], f32, tag="tr", name="ps_rt")
                        ps_r = ps_rt[:, 0:1]
                        nc.tensor.transpose(
                            ps_r, den_sb[:, c * P:(c + 1) * P], identity[0:1, 0:1]
                        )
                        nc.vector.reciprocal(recip_nat[:, c:c + 1], ps_r)
                    # transpose outT back to natural [s, h] and scale by recip
                    for c in range(NC4):
                        ps_o = tr2_ps.tile([P, P], f32, tag="tr")
                        nc.tensor.transpose(
                            ps_o, outT_sb[:, c * P:(c + 1) * P], identity
                        )
                        out_nat = sb2_pool.tile([P, H], f32, tag="out_nat")
                        nc.scalar.activation(
                            out_nat, ps_o, Copy, scale=recip_nat[:, c:c + 1]
                        )
                        row = s_lo + c * P
                        nc.sync.dma_start(out_d[row:row + P, :], out_nat)

    nc.compile()
    _program_cache[key] = nc
    return nc


def kernel(x, wq, wk, wv, drop_u):
    from concourse import bass_utils

    x = np.asarray(x)
    wq = np.asarray(wq)
    wk = np.asarray(wk)
    wv = np.asarray(wv)
    drop_u = np.asarray(drop_u)

    nc = _build_program()
    in_maps = build_in_maps(x, wq, wk, wv, drop_u)
    res = bass_utils.run_bass_kernel_spmd(
        nc, in_maps, core_ids=list(range(B)), trace=False
    )
    return np.stack([res.results[b]["out"] for b in range(B)], axis=0)


def build_in_maps(x, wq, wk, wv, drop_u):
    in_maps = []
    for b in range(B):
        xT = np.ascontiguousarray(x[b].T)
        keepT = np.ascontiguousarray(
            (drop_u[b].T >= np.float32(DROP_P)).astype(np.float16)
        )
        in_maps.append(
            {"xT": xT, "keepT": keepT, "wq": np.asarray(wq), "wk": np.asarray(wk),
             "wv": np.asarray(wv)}
        )
    return in_maps


# revision 10
# speedup vs baseline: 3.2852x; 1.4443x over previous
"""Single-head attention (B=8, S=2048, E=1024, H=128) with softmax + deterministic
dropout, data-parallel over batch across 8 NeuronCores (one batch element per core).

Per-core layout strategy ("transposed attention"):
  - host ships xT = x[b].T           [E, S]  fp16 (contraction dim E on partitions)
  - host ships keepT = keep[b].T     [S, S]  fp16 {0,1} (dropout mask, t-major)
  - qT/kT/vT[h, s] = w.T @ xT        (PE fp16, fp32 PSUM)
  - v natural [t, h] via 16 PE transposes of vT (fp16)
  - attT[t, s] = k[t-chunk] @ qT     (PE fp16; lhsT = kT chunk, rhs = qT slice)
  - expT = exp(attT * E^-0.5)        (ACT, PSUM -> fp16 SBUF)
  - denomT[1, s] += ones.T @ expT    (PE fp16, M=1, accumulated over t-chunks)
  - attd = expT * keepT              (DVE fp16, 2x mode)
  - outT[h, s] += v[t-chunk].T @ attd  (PE fp16, fp32 PSUM accumulation)
  - normalize by 1/(0.9*denom) fused into the final PSUM->SBUF copy (ACT
    per-partition scale) after PE-transposing outT back to natural [s, h].

Precision: fp16 rounding on x/w/q/k contributes only ~3e-5 to the softmax
logits (their absolute scale is ~0.1 after the E^-0.5 scaling); the fp16
value path (v, exp, attd) dominates at ~2-3e-4 L2 on the output, with all
contractions accumulated in fp32 PSUM.
"""

import sys

for _p in ("/opt/trn_rl_repo",):
    if _p not in sys.path:
        sys.path.append(_p)

import numpy as np

B, S, E, H = 8, 2048, 1024, 128
DROP_P = 0.1
P = 128

_program_cache = {}


def _build_program(S=S, E=E):
    key = (S, E)
    if key in _program_cache:
        return _program_cache[key]
    NT = S // P  # t-chunks
    NE = E // P  # e-chunks
    SG = 512     # s-group width (one fp32 PSUM bank)
    NSG = S // SG
    NC4 = SG // P

    import concourse.bass as bass  # noqa: F401
    import concourse.mybir as mybir
    import concourse.tile as tile
    from concourse import bacc
    from concourse.masks import make_identity

    f32 = mybir.dt.float32
    f16 = mybir.dt.float16
    Exp = mybir.ActivationFunctionType.Exp
    Copy = mybir.ActivationFunctionType.Copy
    SCALE = float(E) ** -0.5

    nc = bacc.Bacc("TRN2", target_bir_lowering=False, debug=False)
    xT_d = nc.dram_tensor("xT", [E, S], f16, kind="ExternalInput").ap()
    keepT_d = nc.dram_tensor("keepT", [S, S], f16, kind="ExternalInput").ap()
    wq_d = nc.dram_tensor("wq", [E, H], f16, kind="ExternalInput").ap()
    wk_d = nc.dram_tensor("wk", [E, H], f16, kind="ExternalInput").ap()
    wv_d = nc.dram_tensor("wv", [E, H], f16, kind="ExternalInput").ap()
    out_d = nc.dram_tensor("out", [S, H], f32, kind="ExternalOutput").ap()

    xT_r = xT_d.rearrange("(eo p) s -> p eo s", p=P)
    w_rs = [w.rearrange("(eo p) h -> p eo h", p=P) for w in (wq_d, wk_d, wv_d)]
    # keepT viewed as [p, t_chunk, s] so one DMA loads a whole s-group column
    keepT_r = keepT_d.rearrange("(to p) s -> p to s", p=P)

    with tile.TileContext(nc) as tc:
        with (
            tc.tile_pool(name="consts", bufs=1) as consts,
            tc.tile_pool(name="xw", bufs=1) as xw_pool,
            tc.tile_pool(name="qkv", bufs=1) as qkv_pool,
        ):
            identity = consts.tile([P, P], f32)
            make_identity(nc, identity)
            identity16 = consts.tile([P, P], f16)
            nc.any.tensor_copy(identity16, identity)
            ones_t = consts.tile([P, 1], f16)
            nc.vector.memset(ones_t, 1.0)

            # -------- load x^T and weights --------
            xT_sb = xw_pool.tile([P, NE, S], f16)
            for e in range(NE):
                nc.sync.dma_start(xT_sb[:, e, :], xT_r[:, e, :])
            w_sb = xw_pool.tile([P, 3, NE, H], f16)
            for j in range(3):
                for e in range(NE):
                    nc.sync.dma_start(w_sb[:, j, e, :], w_rs[j][:, e, :])

            # -------- projections: qT/kT [H, S] f16; vT -> v natural f16 --------
            qkT_sb = qkv_pool.tile([P, 2, S], f16)  # [h, (q|k), s]
            vT_sb = qkv_pool.tile([P, S], f16)
            v_sb = qkv_pool.tile([P, NT, H], f16)   # v natural: [t_in, t_chunk, h]
            with (
                tc.tile_pool(name="proj_ps", bufs=3, space="PSUM") as proj_ps,
                tc.tile_pool(name="tr_ps", bufs=2, space="PSUM") as tr_ps,
            ):
                for j in range(3):
                    for c in range(S // SG):
                        ps = proj_ps.tile([P, SG], f32, tag="proj")
                        for e in range(NE):
                            nc.tensor.matmul(
                                ps,
                                w_sb[:, j, e, :],
                                xT_sb[:, e, c * SG:(c + 1) * SG],
                                start=(e == 0),
                                stop=(e == NE - 1),
                            )
                        if j < 2:
                            nc.any.tensor_copy(qkT_sb[:, j, c * SG:(c + 1) * SG], ps)
                        else:
                            nc.any.tensor_copy(vT_sb[:, c * SG:(c + 1) * SG], ps)
                # v natural via PE transpose of vT (fp16, exact for fp16 data)
                for t in range(NT):
                    ps_v = tr_ps.tile([P, P], f16, tag="vtr")
                    nc.tensor.transpose(
                        ps_v, vT_sb[:, t * P:(t + 1) * P], identity16
                    )
                    nc.any.tensor_copy(v_sb[:, t, :], ps_v)

            # -------- main attention loop over s-groups --------
            with (
                tc.tile_pool(name="att_ps", bufs=3, space="PSUM") as att_ps,
                tc.tile_pool(name="out_ps", bufs=2, space="PSUM") as out_ps,
                tc.tile_pool(name="den_ps", bufs=1, space="PSUM") as den_ps,
                tc.tile_pool(name="tr2_ps", bufs=2, space="PSUM") as tr2_ps,
                tc.tile_pool(name="keep_pool", bufs=2) as keep_pool,
                tc.tile_pool(name="sb", bufs=3) as sb_pool,
                tc.tile_pool(name="sb2", bufs=2) as sb2_pool,
            ):
                for sg in range(NSG):
                    s_lo = sg * SG
                    s_sl = slice(s_lo, s_lo + SG)
                    # one big strided DMA: dropout mask for every t at this s-group
                    keep_sg = keep_pool.tile([P, NT, SG], f16, tag="keep")
                    nc.sync.dma_start(keep_sg, keepT_r[:, :, s_sl])
                    psum_out = out_ps.tile([P, SG], f32, tag="out")
                    psum_den = den_ps.tile([1, SG], f32, tag="den")
                    expTs = {}
                    attds = {}

                    def emit_front(t, s_sl=s_sl, keep_sg=keep_sg,
                                   expTs=expTs, attds=attds):
                        psum_att = att_ps.tile([P, SG], f32, tag="att", name=f"att{t}")
                        nc.tensor.matmul(
                            psum_att,
                            qkT_sb[:, 1, t * P:(t + 1) * P],  # kT chunk [H, 128]
                            qkT_sb[:, 0, s_sl],               # qT slice [H, 512]
                            start=True,
                            stop=True,
                        )
                        expT = sb_pool.tile([P, SG], f16, tag="exp", name=f"exp{t}")
                        nc.scalar.activation(expT, psum_att, Exp, scale=SCALE)
                        attd = sb_pool.tile([P, SG], f16, tag="attd", name=f"attd{t}")
                        nc.vector.tensor_mul(out=attd, in0=expT, in1=keep_sg[:, t, :])
                        expTs[t] = expT
                        attds[t] = attd

                    def emit_back(t, psum_den=psum_den, psum_out=psum_out,
                                  expTs=expTs, attds=attds):
                        nc.tensor.matmul(
                            psum_den,
                            ones_t,
                            expTs.pop(t),
                            start=(t == 0),
                            stop=(t == NT - 1),
                        )
                        nc.tensor.matmul(
                            psum_out,
                            v_sb[:, t, :],
                            attds.pop(t),
                            start=(t == 0),
                            stop=(t == NT - 1),
                        )

                    # software pipeline: back-stage ops run one iteration behind
                    # the att matmul so PE never waits on ACT/DVE results.
                    for t in range(NT):
                        emit_front(t)
                        if t >= 1:
                            emit_back(t - 1)
                    emit_back(NT - 1)

                    # denominator -> natural-layout 1/(0.9*den) chunks [s_in, 1]
                    den_sb = sb2_pool.tile([1, SG], f32, tag="den_sb")
                    nc.scalar.mul(den_sb, psum_den, 1.0 - DROP_P)
                    outT_sb = sb2_pool.tile([P, SG], f32, tag="outT")
                    nc.any.tensor_copy(outT_sb, psum_out)
                    recip_nat = sb2_pool.tile([P, NC4], f32, tag="recip")
                    for c in range(NC4):
                        ps_rt = tr2_ps.tile([P, P], f32, tag="tr", name="ps_rt")
                        ps_r = ps_rt[:, 0:1]
                        nc.tensor.transpose(
                            ps_r, den_sb[:, c * P:(c + 1) * P], identity[0:1, 0:1]
                        )
                        nc.vector.reciprocal(recip_nat[:, c:c + 1], ps_r)
                    # transpose outT back to natural [s, h] and scale by recip
                    for c in range(NC4):
                        ps_o = tr2_ps.tile([P, P], f32, tag="tr")
                        nc.tensor.transpose(
                            ps_o, outT_sb[:, c * P:(c + 1) * P], identity
                        )
                        out_nat = sb2_pool.tile([P, H], f32, tag="out_nat")
                        nc.scalar.activation(
                            out_nat, ps_o, Copy, scale=recip_nat[:, c:c + 1]
                        )
                        row = s_lo + c * P
                        nc.sync.dma_start(out_d[row:row + P, :], out_nat)

    nc.compile()
    _program_cache[key] = nc
    return nc


def kernel(x, wq, wk, wv, drop_u):
    from concourse import bass_utils

    x = np.asarray(x)
    wq = np.asarray(wq)
    wk = np.asarray(wk)
    wv = np.asarray(wv)
    drop_u = np.asarray(drop_u)

    nc = _build_program()
    in_maps = build_in_maps(x, wq, wk, wv, drop_u)
    res = bass_utils.run_bass_kernel_spmd(
        nc, in_maps, core_ids=list(range(B)), trace=False
    )
    return np.stack([res.results[b]["out"] for b in range(B)], axis=0)


def build_in_maps(x, wq, wk, wv, drop_u):
    wq16 = np.asarray(wq).astype(np.float16)
    wk16 = np.asarray(wk).astype(np.float16)
    wv16 = np.asarray(wv).astype(np.float16)
    in_maps = []
    for b in range(B):
        xT = np.ascontiguousarray(x[b].T).astype(np.float16)
        keepT = np.ascontiguousarray(
            (drop_u[b].T >= np.float32(DROP_P)).astype(np.float16)
        )
        in_maps.append(
            {"xT": xT, "keepT": keepT, "wq": wq16, "wk": wk16, "wv": wv16}
        )
    return in_maps
